# revision 1
# baseline (speedup 1.0000x reference)
"""Haar 3D wavelet transform (2x2x2 stride-2 conv, 8 sign filters) on 8 trn2 cores.

Input  x: (2, 3, 33, 512, 512) f32, w: (8, 1, 2, 2, 2) f32 (separable +-scale Haar).
Output:   (2, 24, 17, 256, 256) f32.

Data parallel over (b, c, t_out) "units", 102 units padded to 13 per core.
The host pre-permutes each unit's two input frames (x[2t-1], x[2t]; frame 0
replicated for t=0) into the exact SBUF tile layout
    partition p = dt*64 + dh*32 + g, free f = j*512 + w
      <-> frame dt, row (g*8+j)*2 + dh, col w
so the device does ONE contiguous 2MB DMA per unit.  On-chip:
  - (t, h) Haar butterfly = 8 stationary 128x128 block matmuls on TensorE:
        psum[q*32+g, f] = sum_{dt,dh} w0[q][dt,dh] * x[dt*64+dh*32+g, f]
  - ScalarE evacuates PSUM -> SBUF (HW allows only one PSUM operand on DVE).
  - VectorE does the w-pairing (even/odd col sum/diff, stride-2 reads).
  - ONE contiguous 2MB DMA stores the unit result; host un-permutes.
"""

import numpy as np

N_CORES = 8
B, C, T_IN, H, W = 2, 3, 33, 512, 512
T_OUT, HO, WO = 17, 256, 256
UNITS = [(b, c, t) for b in range(B) for c in range(C) for t in range(T_OUT)]
N_UNITS_PER_CORE = (len(UNITS) + N_CORES - 1) // N_CORES  # 13


def _build_nc(n_units, legalize=True):
    import concourse.bass as bass
    import concourse.mybir as mybir
    from concourse.tile import TileContext

    nc = bass.Bass()
    xin = nc.declare_dram_parameter(
        "xin", [n_units, 128, 4096], mybir.dt.float32, isOutput=False)
    wmat = nc.declare_dram_parameter(
        "wmat", [128, 128], mybir.dt.float32, isOutput=False)
    yout = nc.declare_dram_parameter(
        "yout", [n_units, 128, 4096], mybir.dt.float32, isOutput=True)

    with TileContext(nc) as tc:
        with (
            tc.tile_pool(name="const", bufs=1) as cpool,
            tc.tile_pool(name="xpool", bufs=3) as xpool,
            tc.tile_pool(name="spool", bufs=4) as spool,
            tc.tile_pool(name="rpool", bufs=3) as rpool,
            tc.tile_pool(name="ppool", bufs=2, space="PSUM") as ppool,
        ):
            wt = cpool.tile([128, 128], mybir.dt.float32)
            nc.sync.dma_start(out=wt[:], in_=wmat[:])

            for u in range(n_units):
                xt = xpool.tile([128, 4096], mybir.dt.float32)
                nc.sync.dma_start(out=xt[:], in_=xin[u])

                rt = rpool.tile([128, 4096], mybir.dt.float32)
                for hh in range(2):  # half-unit = 4 PSUM banks -> ping-pong
                    pt = ppool.tile([128, 2048], mybir.dt.float32)
                    for m in range(4):
                        f0 = hh * 2048 + m * 512
                        nc.tensor.matmul(
                            pt[:, m * 512:(m + 1) * 512],
                            lhsT=wt[:],
                            rhs=xt[:, f0:f0 + 512],
                            start=True, stop=True)

                    # PSUM -> SBUF on ScalarE (single-PSUM-operand rule),
                    # then even/odd column sum/diff on VectorE.
                    st = spool.tile([128, 2048], mybir.dt.float32)
                    nc.scalar.copy(st[:], pt[:])
                    # staged free = jj*512 + 2*wp + e
                    # result free = hh*2048 + fw*1024 + jj*256 + wp
                    sv = st[:].rearrange("p (jj wp e) -> p jj wp e", jj=4, wp=256, e=2)
                    rv = rt[:, hh * 2048:(hh + 1) * 2048].rearrange(
                        "p (fw jj wp) -> p fw jj wp", fw=2, jj=4, wp=256)
                    nc.vector.tensor_add(
                        out=rv[:, 0], in0=sv[:, :, :, 0], in1=sv[:, :, :, 1])
                    nc.vector.tensor_sub(
                        out=rv[:, 1], in0=sv[:, :, :, 0], in1=sv[:, :, :, 1])

                nc.scalar.dma_start(out=yout[u], in_=rt[:])

    if legalize:
        _legalize_waits(nc)
    return nc


def _legalize_waits(nc, limit=1):
    """walrus codegen rejects instructions carrying more than ~1 sem wait
    (e.g. Matmult's LoadWeights slot).  Move excess waits onto NoOp
    instructions inserted just before the instruction on the same engine
    queue -- semantically identical (all waits still precede execution)."""
    import bass_rust

    fn = nc.m.functions[0]
    lastblk = fn.blocks[-1]
    eng_ns = {
        "PE": nc.tensor, "DVE": nc.vector, "Activation": nc.scalar,
        "SP": nc.sync, "Pool": nc.gpsimd,
    }
    # NoOp codegen requires >=1 sem update. Give each engine its own dummy
    # sem (ids picked from the top of the 150..255 HW range, skipping any id
    # already referenced) so no counting or cross-proc rule is disturbed.
    used_ids = set()
    for blk in fn.blocks:
        for inst in blk.instructions:
            si = getattr(inst, "sync_info", None)
            if si is None:
                continue
            for w in si.on_wait:
                used_ids.add(w.id)
            for upd in si.on_update:
                used_ids.add(upd.id)
    avail = [i for i in range(255, 149, -1) if i not in used_ids]
    eng_upd = {}
    for k, en in enumerate(["PE", "DVE", "Activation", "SP", "Pool"]):
        eng_upd[en] = bass_rust.SyncUpdate(
            sync_type="semaphore", id=avail[k], ant_name=f"waitnop_{en}",
            update_mode="sem-inc", update_value=1, update_reg=None)

    def copy_wait(w):
        return bass_rust.SyncWait(
            sync_type=w.sync_type, id=w.id, ant_name=w.ant_name,
            wait_mode=w.wait_mode, wait_value=w.wait_value, wait_reg=w.wait_reg)

    def make_nop(engine_name, waits):
        ns = eng_ns[engine_name]
        ns.nop(hint="waitcarrier")
        nop = lastblk.instructions.pop()
        raw = getattr(nop, "inst", nop)
        raw.sync_info = bass_rust.SyncInfo(
            on_wait=[copy_wait(w) for w in waits],
            on_update=[eng_upd[engine_name]])
        return raw

    for blk in fn.blocks:
        insts = blk.instructions
        i = 0
        while i < len(insts):
            inst = insts[i]
            ty = type(inst).__name__
            si = getattr(inst, "sync_info", None)
            if (ty not in ("InstEventSemaphore", "InstNoOp")
                    and si is not None and len(si.on_wait) > limit):
                ename = str(inst.engine).split(".")[-1]
                waits = [copy_wait(w) for w in si.on_wait]
                upds = list(si.on_update)
                extra, keep = waits[:-limit], waits[-limit:]
                for w in extra:
                    insts.insert(i, make_nop(ename, [w]))
                    i += 1
                inst.sync_info = bass_rust.SyncInfo(
                    on_wait=keep, on_update=upds)
            i += 1


def _make_wmat(w):
    """128x128 stationary matrix for the (t,h) butterfly; asserts w is
    dw-separable with +-1 sign (true for the Haar module)."""
    w = np.asarray(w, dtype=np.float32).reshape(8, 2, 2, 2)
    c0, c1 = w[:, :, :, 0], w[:, :, :, 1]
    for q in range(4):
        assert np.allclose(c0[2 * q], c0[2 * q + 1], atol=1e-6)
        assert np.allclose(c1[2 * q], c0[2 * q], atol=1e-6)
        assert np.allclose(c1[2 * q + 1], -c0[2 * q + 1], atol=1e-6)
    wm = np.zeros((128, 128), dtype=np.float32)
    for q in range(4):
        for dt in range(2):
            for dh in range(2):
                v = c0[2 * q, dt, dh]
                for g in range(32):
                    wm[dt * 64 + dh * 32 + g, q * 32 + g] = v
    return wm


def _pack_input(frames):
    """(nu, 2, 512, 512) frame pairs -> (nu, 128, 4096) device layout."""
    nu = frames.shape[0]
    v = frames.reshape(nu, 2, 32, 8, 2, 512)        # u dt g j dh w
    v = v.transpose(0, 1, 4, 2, 3, 5)               # u dt dh g j w
    return np.ascontiguousarray(v).reshape(nu, 128, 4096)


def _unpack_output(yo):
    """(nu, 128, 4096) device layout -> (nu, 8, 256, 256) filter planes."""
    nu = yo.shape[0]
    v = yo.reshape(nu, 4, 32, 2, 2, 4, 256)         # u q g hh fw jj wp
    v = v.transpose(0, 1, 4, 2, 3, 5, 6)            # u q fw g hh jj wp
    return np.ascontiguousarray(v).reshape(nu, 8, 256, 256)


LAST_RESULT = None


def kernel(x, w):
    import os
    from concourse.bass_utils import run_bass_kernel_spmd

    x = np.asarray(x, dtype=np.float32)
    wm = _make_wmat(w)
    nu = N_UNITS_PER_CORE

    # ---- shard: core m takes UNITS[m::8], padded to nu with repeats ----
    in_maps = []
    core_units = []
    for m in range(N_CORES):
        us = UNITS[m::N_CORES]
        core_units.append(us)
        frames = np.empty((nu, 2, H, W), dtype=np.float32)
        for s in range(nu):
            b, c, t = us[s % len(us)]
            frames[s, 0] = x[b, c, max(2 * t - 1, 0)]
            frames[s, 1] = x[b, c, 2 * t]
        in_maps.append({"xin": _pack_input(frames), "wmat": wm})

    nc = _build_nc(nu)
    kw = {}
    if os.environ.get("KERNEL_PROFILE") == "1":
        kw = dict(trace=True, tmpdir=os.environ.get("KERNEL_PROFILE_DIR"))
    res = run_bass_kernel_spmd(nc, in_maps, core_ids=list(range(N_CORES)), **kw)
    global LAST_RESULT
    LAST_RESULT = res

    # ---- unshard ----
    out = np.empty((B, 8 * C, T_OUT, HO, WO), dtype=np.float32)
    for m in range(N_CORES):
        yo = _unpack_output(np.asarray(res.results[m]["yout"]))
        for s, (b, c, t) in enumerate(core_units[m]):
            for k in range(8):
                out[b, 3 * k + c, t] = yo[s, k]
    return out


if __name__ == "__main__":
    x = np.random.randn(B, C, T_IN, H, W).astype(np.float32)
    SCALE = 0.3536
    flags = np.array([[0, 0, 0], [0, 0, 1], [0, 1, 0], [0, 1, 1],
                      [1, 0, 0], [1, 0, 1], [1, 1, 0], [1, 1, 1]])
    t, h, ww = np.meshgrid(np.arange(2), np.arange(2), np.arange(2), indexing="ij")
    sign = (-1.0) ** (flags[:, 0, None, None, None] * t
                      + flags[:, 1, None, None, None] * h
                      + flags[:, 2, None, None, None] * ww)
    wf = (SCALE * sign).reshape(8, 1, 2, 2, 2).astype(np.float32)
    y = kernel(x, wf)
    print(y.shape, y.dtype)



# revision 3
# speedup vs baseline: 1.9531x; 1.9531x over previous
"""Haar 3D wavelet transform (2x2x2 stride-2 conv, 8 sign filters) on 8 trn2 cores.

Input  x: (2, 3, 33, 512, 512) f32, w: (8, 1, 2, 2, 2) f32.
Output:   (2, 24, 17, 256, 256) f32.

Memory-bound problem -> move bytes as bf16 (tolerance 2e-2, bf16 round-trip
costs ~4e-3).  The host packs each (b, c, t_out) unit's two input frames
(x[2t-1], x[2t]; frame 0 replicated for t=0) so that the 8 taps of every
2x2x2 block land in 8 different SBUF partitions:

    partition p = dt*64 + dh*32 + dw*16 + g      (g = row-group 0..15)
    free      f = j*256 + wp                      (row ho = g*16 + j, col wp)

Then ONE stationary 128x128 matrix computes all 8 filter outputs per block:

    psum[k*16 + g, f] = sum_{dt,dh,dw} w[k,0,dt,dh,dw] * x[(dt,dh,dw,g), f]

i.e. the whole transform is a per-column 128x128 matmul.  All 102*4096
columns are independent, so they are split exactly 52224 per core (no
padding waste).  On-chip per tile: DMA-in (sync) -> 6 matmuls (PE, bf16)
-> PSUM->SBUF cast copies (split scalar/vector) -> DMA-out (scalar).
"""

import numpy as np

N_CORES = 8
B, C, T_IN, H, W = 2, 3, 33, 512, 512
T_OUT, HO, WO = 17, 256, 256
N_UNITS = B * C * T_OUT                      # 102
UNIT_COLS = 4096                             # free columns per unit
COLS_TOTAL = N_UNITS * UNIT_COLS             # 417792
COLS_PER_CORE = COLS_TOTAL // N_CORES        # 52224
F_TILE = 3072                                # 6 x 512-wide matmul chunks
N_TILES = COLS_PER_CORE // F_TILE            # 17


def _build_nc(legalize=True):
    import concourse.bass as bass
    import concourse.mybir as mybir
    from concourse.tile import TileContext

    nc = bass.Bass()
    xin = nc.declare_dram_parameter(
        "xin", [128, COLS_PER_CORE], mybir.dt.bfloat16, isOutput=False)
    wmat = nc.declare_dram_parameter(
        "wmat", [128, 128], mybir.dt.bfloat16, isOutput=False)
    yout = nc.declare_dram_parameter(
        "yout", [128, COLS_PER_CORE], mybir.dt.bfloat16, isOutput=True)

    with TileContext(nc) as tc:
        with (
            tc.tile_pool(name="const", bufs=1) as cpool,
            tc.tile_pool(name="xpool", bufs=3) as xpool,
            tc.tile_pool(name="ypool", bufs=3) as ypool,
            tc.tile_pool(name="ppool", bufs=8, space="PSUM") as ppool,
        ):
            wt = cpool.tile([128, 128], mybir.dt.bfloat16)
            nc.sync.dma_start(out=wt[:], in_=wmat[:])

            for i in range(N_TILES):
                c0 = i * F_TILE
                xt = xpool.tile([128, F_TILE], mybir.dt.bfloat16)
                nc.sync.dma_start(out=xt[:], in_=xin[:, c0:c0 + F_TILE])

                yt = ypool.tile([128, F_TILE], mybir.dt.bfloat16)
                for m in range(F_TILE // 512):
                    f0 = m * 512
                    pt = ppool.tile([128, 512], mybir.dt.float32)
                    nc.tensor.matmul(
                        pt[:], lhsT=wt[:], rhs=xt[:, f0:f0 + 512],
                        start=True, stop=True)
                    if m % 2 == 0:
                        nc.scalar.copy(yt[:, f0:f0 + 512], pt[:])
                    else:
                        nc.vector.tensor_copy(yt[:, f0:f0 + 512], pt[:])

                nc.scalar.dma_start(out=yout[:, c0:c0 + F_TILE], in_=yt[:])

    if legalize:
        _legalize_waits(nc)
    return nc


def _legalize_waits(nc, limit=1):
    """walrus codegen rejects instructions carrying more than ~1 sem wait
    (e.g. Matmult's LoadWeights slot).  Move excess waits onto NoOp
    instructions inserted just before the instruction on the same engine
    queue -- semantically identical (all waits still precede execution)."""
    import bass_rust

    fn = nc.m.functions[0]
    lastblk = fn.blocks[-1]
    eng_ns = {
        "PE": nc.tensor, "DVE": nc.vector, "Activation": nc.scalar,
        "SP": nc.sync, "Pool": nc.gpsimd,
    }
    # NoOp codegen requires >=1 sem update. Give each engine its own dummy
    # sem (ids picked from the top of the 150..255 HW range, skipping any id
    # already referenced) so no counting or cross-proc rule is disturbed.
    used_ids = set()
    for blk in fn.blocks:
        for inst in blk.instructions:
            si = getattr(inst, "sync_info", None)
            if si is None:
                continue
            for w in si.on_wait:
                used_ids.add(w.id)
            for upd in si.on_update:
                used_ids.add(upd.id)
    avail = [i for i in range(255, 149, -1) if i not in used_ids]
    eng_upd = {}
    for k, en in enumerate(["PE", "DVE", "Activation", "SP", "Pool"]):
        eng_upd[en] = bass_rust.SyncUpdate(
            sync_type="semaphore", id=avail[k], ant_name=f"waitnop_{en}",
            update_mode="sem-inc", update_value=1, update_reg=None)

    def copy_wait(w):
        return bass_rust.SyncWait(
            sync_type=w.sync_type, id=w.id, ant_name=w.ant_name,
            wait_mode=w.wait_mode, wait_value=w.wait_value, wait_reg=w.wait_reg)

    def make_nop(engine_name, waits):
        ns = eng_ns[engine_name]
        ns.nop(hint="waitcarrier")
        nop = lastblk.instructions.pop()
        raw = getattr(nop, "inst", nop)
        raw.sync_info = bass_rust.SyncInfo(
            on_wait=[copy_wait(w) for w in waits],
            on_update=[eng_upd[engine_name]])
        return raw

    for blk in fn.blocks:
        insts = blk.instructions
        i = 0
        while i < len(insts):
            inst = insts[i]
            ty = type(inst).__name__
            si = getattr(inst, "sync_info", None)
            if (ty not in ("InstEventSemaphore", "InstNoOp")
                    and si is not None and len(si.on_wait) > limit):
                ename = str(inst.engine).split(".")[-1]
                waits = [copy_wait(w) for w in si.on_wait]
                upds = list(si.on_update)
                extra, keep = waits[:-limit], waits[-limit:]
                for w in extra:
                    insts.insert(i, make_nop(ename, [w]))
                    i += 1
                inst.sync_info = bass_rust.SyncInfo(
                    on_wait=keep, on_update=upds)
            i += 1


def _make_wmat(w):
    """128x128 stationary butterfly: wm[p, q] with p = dt*64+dh*32+dw*16+g,
    q = k*16+g, value w[k,0,dt,dh,dw].  Fully general in w."""
    w = np.asarray(w, dtype=np.float32).reshape(8, 2, 2, 2)
    wm = np.zeros((128, 128), dtype=np.float32)
    g = np.arange(16)
    for k in range(8):
        for dt in range(2):
            for dh in range(2):
                for dw in range(2):
                    wm[dt * 64 + dh * 32 + dw * 16 + g, k * 16 + g] = \
                        w[k, dt, dh, dw]
    return wm


def _pack_input(x16):
    """(B,C,T_IN,512,512) bf16 -> (128, COLS_TOTAL) device column layout."""
    t = np.arange(T_OUT)
    t0 = np.maximum(2 * t - 1, 0)
    t1 = 2 * t
    fp = np.stack([x16[:, :, t0], x16[:, :, t1]], axis=3)  # b c t dt 512 512
    v = fp.reshape(N_UNITS, 2, 16, 16, 2, 256, 2)          # u dt g j dh wp dw
    v = v.transpose(0, 1, 4, 6, 2, 3, 5)                   # u dt dh dw g j wp
    p = v.reshape(N_UNITS, 128, UNIT_COLS)
    return p.transpose(1, 0, 2).reshape(128, COLS_TOTAL)


def _unpack_output(yg):
    """(128, COLS_TOTAL) bf16 device layout -> (2, 24, 17, 256, 256) f32."""
    q = yg.reshape(128, N_UNITS, UNIT_COLS).transpose(1, 0, 2)
    planes = q.reshape(N_UNITS, 8, HO, WO)                 # u k (g j)=ho wp
    out = planes.reshape(B, C, T_OUT, 8, HO, WO)
    return np.ascontiguousarray(
        out.transpose(0, 3, 1, 2, 4, 5)).reshape(
        B, 8 * C, T_OUT, HO, WO).astype(np.float32)


LAST_RESULT = None


def kernel(x, w):
    import os
    import ml_dtypes
    from concourse.bass_utils import run_bass_kernel_spmd

    bf16 = ml_dtypes.bfloat16
    x16 = np.asarray(x, dtype=np.float32).astype(bf16)
    wm = _make_wmat(w).astype(bf16)

    g = _pack_input(x16)
    in_maps = []
    for m in range(N_CORES):
        sl = np.ascontiguousarray(
            g[:, m * COLS_PER_CORE:(m + 1) * COLS_PER_CORE])
        in_maps.append({"xin": sl, "wmat": wm})

    nc = _build_nc()
    kw = {}
    if os.environ.get("KERNEL_PROFILE") == "1":
        kw = dict(trace=True, tmpdir=os.environ.get("KERNEL_PROFILE_DIR"))
    res = run_bass_kernel_spmd(nc, in_maps, core_ids=list(range(N_CORES)), **kw)
    global LAST_RESULT
    LAST_RESULT = res

    yg = np.concatenate(
        [np.asarray(res.results[m]["yout"]) for m in range(N_CORES)], axis=1)
    return _unpack_output(yg)


if __name__ == "__main__":
    x = np.random.randn(B, C, T_IN, H, W).astype(np.float32)
    SCALE = 0.3536
    flags = np.array([[0, 0, 0], [0, 0, 1], [0, 1, 0], [0, 1, 1],
                      [1, 0, 0], [1, 0, 1], [1, 1, 0], [1, 1, 1]])
    t, h, ww = np.meshgrid(np.arange(2), np.arange(2), np.arange(2), indexing="ij")
    sign = (-1.0) ** (flags[:, 0, None, None, None] * t
                      + flags[:, 1, None, None, None] * h
                      + flags[:, 2, None, None, None] * ww)
    wf = (SCALE * sign).reshape(8, 1, 2, 2, 2).astype(np.float32)
    y = kernel(x, wf)
    print(y.shape, y.dtype)
